# revision 3
# baseline (speedup 1.0000x reference)
"""MultiHeadAttention TRN2 Bass kernel (v4, K/V pair exchange, promoted).

B=4, S=2048, D=1024, H=16, head_dim=64. Q,K,V all derived from `query`.

Sharding: 8 cores = (batch 0..3) x (seq half 0..1). Each core receives its
batch's full 2048 tokens (rolled local-first), computes K,V for all 2048
(duplicated within the pair), attention + output projection for its local
1024 queries.

v3 = baseline structure with:
  - Host pre-transposed bf16 inputs (qT/wT/woT): no on-device transposes,
    casts, or DRAM weight bounce.
  - exp split across ACT (exact) and DVE (Schraudolph int16-bitcast
    approximation, ~2% rms) to break the ACT exp bottleneck.
  - AV in the baseline orientation (stationary [v|1] tiles, 512-wide
    moving P) — full moving-size amortization of PE stationary loads.
  - Normalization via E-matrix broadcast matmul (baseline scheme).
"""
import os
import sys

sys.path.insert(0, "/opt/trn_rl_repo")

import numpy as np
import ml_dtypes
import concourse.bacc as bacc
import concourse.tile as tile
import concourse.mybir as mybir
from concourse.bass_utils import run_bass_kernel_spmd

F32 = mybir.dt.float32
F32R = mybir.dt.float32r
BF16 = mybir.dt.bfloat16
I16 = mybir.dt.int16
AF = mybir.ActivationFunctionType
ALU = mybir.AluOpType

B, S, D = 4, 2048, 1024
H, HD = 16, 64
SLOC = 1024
N_CORES = 8

SCH_A = float(184.6627 / 8.0)
SCH_B = 16250.5

_CACHE = {}


def _build(reps=None):
    if reps is None:
        reps = int(os.environ.get("KERNEL_REPS", "1"))
    nc = bacc.Bacc("TRN2", target_bir_lowering=False, debug=False,
                   num_devices=N_CORES)
    qT_in = nc.dram_tensor("qT", [D, SLOC], BF16, kind="ExternalInput")
    wT_in = nc.dram_tensor("wT", [D, 3 * D], BF16, kind="ExternalInput")
    woT_in = nc.dram_tensor("woT", [D, D], BF16, kind="ExternalInput")
    b_in = nc.dram_tensor("b_out", [D], F32, kind="ExternalInput")
    out = nc.dram_tensor("out", [SLOC, D], F32, kind="ExternalOutput")

    class EngPick:
        def __init__(self, pattern):
            self.pattern = pattern
            self.i = 0

        def __call__(self):
            c = self.pattern[self.i % len(self.pattern)]
            self.i += 1
            return c

    exp_pick = EngPick(os.environ.get("EXP_PATTERN", "FW"))
    cp_pick = EngPick(os.environ.get("COPY_PATTERN", "AV"))

    with tile.TileContext(nc) as tc:
        with tc.tile_pool(name="persist", bufs=1) as persist:
            for _rep in range(reps):
                ones_row = persist.tile([1, 128], BF16, tag="ones_row")
                nc.any.memset(ones_row[:], 1.0)
                ones_col = persist.tile([128, 1], F32, tag="ones_col")
                nc.any.memset(ones_col[:], 1.0)
                bias_f32 = persist.tile([1, D], F32, tag="bias_f32")
                nc.sync.dma_start(bias_f32[:], b_in[:].unsqueeze(0))
                bias_bf = persist.tile([1, D], BF16, tag="bias_bf")
                nc.vector.tensor_copy(bias_bf[:], bias_f32[:])

                # E matrix for partition-broadcast of the two recip rows
                zeros_f32 = persist.tile([128, 512], F32, tag="zeros_f32")
                nc.any.memset(zeros_f32[:], 0.0)
                E = persist.tile([128, 128], F32R, tag="E")
                nc.vector.tensor_copy(E[:], zeros_f32[:, 0:128])
                nc.vector.tensor_copy(
                    E[64:65, 0:64],
                    ones_col[64:65, 0:1].to_broadcast((1, 64)))
                nc.vector.tensor_copy(
                    E[0:1, 64:128],
                    ones_col[0:1, 0:1].to_broadcast((1, 64)))
                R_tiles = [persist.tile([128, 512], F32R, tag=f"R{i}",
                                        name=f"R{i}") for i in range(2)]
                for Rt in R_tiles:
                    nc.vector.tensor_copy(Rt[:], zeros_f32[:])

                qTp = [persist.tile([128, SLOC], BF16, tag=f"qTp{i}",
                                    name=f"qTp{i}") for i in range(8)]
                kT = [persist.tile([128, S], BF16, tag=f"kT{i}",
                                   name=f"kT{i}") for i in range(8)]
                vte = [persist.tile([128, 8, 65], BF16, tag=f"vte{t}",
                                    name=f"vte{t}") for t in range(16)]
                vto = [persist.tile([128, 8, 128], BF16, tag=f"vto{t}",
                                    name=f"vto{t}") for t in range(16)]
                woT = [persist.tile([128, D], BF16, tag=f"woT{i}",
                                    name=f"woT{i}") for i in range(8)]

                for d in range(8):
                    nc.sync.dma_start(woT[d][:],
                                      woT_in[d * 128:(d + 1) * 128, :])
                for t in range(16):
                    nc.gpsimd.memset(vte[t][:, :, 64:65], 1.0)
                    nc.gpsimd.memset(vto[t][:, :, 0:1], 1.0)
                    nc.gpsimd.memset(vto[t][:, :, 1:64], 0.0)

                # ============ projections ============
                with (
                    tc.tile_pool(name="qt", bufs=1) as qt_pool,
                    tc.tile_pool(name="wt", bufs=1) as wt_pool,
                    tc.tile_pool(name="b_ps", bufs=3, space="PSUM") as b_ps,
                ):
                    qT = [qt_pool.tile([128, SLOC], BF16, tag=f"qT{i}",
                                       name=f"qT{i}") for i in range(8)]
                    wT = [wt_pool.tile([128, 3 * D], BF16, tag=f"wT{i}",
                                       name=f"wT{i}") for i in range(8)]
                    for d in range(8):
                        nc.sync.dma_start(qT[d][:],
                                          qT_in[d * 128:(d + 1) * 128, :])
                        nc.sync.dma_start(wT[d][:],
                                          wT_in[d * 128:(d + 1) * 128, :])

                    # V projection: out [tok, n] (local 8 chunks)
                    for t in range(8):
                        for nf in range(2):
                            ps = b_ps.tile([128, 512], F32, tag="proj")
                            for d in range(8):
                                nc.tensor.matmul(
                                    ps[:],
                                    qT[d][:, t * 128:(t + 1) * 128],
                                    wT[d][:, 2 * D + nf * 512:
                                          2 * D + (nf + 1) * 512],
                                    start=(d == 0), stop=(d == 7))
                            ps3 = ps[:].rearrange("p (j x) -> p j x", x=64)
                            hp0 = 4 * nf
                            for dst, src in (
                                (vte[t][:, hp0:hp0 + 4, 0:64],
                                 ps3[:, 0:8:2, :]),
                                (vto[t][:, hp0:hp0 + 4, 64:128],
                                 ps3[:, 1:8:2, :]),
                            ):
                                if cp_pick() == "A":
                                    nc.scalar.activation(dst, src, AF.Copy)
                                else:
                                    nc.vector.tensor_copy(dst, src)

                    # K projection: out [n, tok] (local cols 0:1024)
                    for fc in range(8):
                        for qc in range(2):
                            ps = b_ps.tile([128, 512], F32, tag="proj")
                            for d in range(8):
                                nc.tensor.matmul(
                                    ps[:],
                                    wT[d][:, D + fc * 128:D + (fc + 1) * 128],
                                    qT[d][:, qc * 512:(qc + 1) * 512],
                                    start=(d == 0), stop=(d == 7))
                            dst = kT[fc][:, qc * 512:(qc + 1) * 512]
                            if cp_pick() == "A":
                                nc.scalar.activation(dst, ps[:], AF.Copy)
                            else:
                                nc.vector.tensor_copy(dst, ps[:])

                    # Q projection (local 1024)
                    for fc in range(8):
                        for qc in range(2):
                            ps = b_ps.tile([128, 512], F32, tag="proj")
                            for d in range(8):
                                nc.tensor.matmul(
                                    ps[:],
                                    wT[d][:, fc * 128:(fc + 1) * 128],
                                    qT[d][:, qc * 512:(qc + 1) * 512],
                                    start=(d == 0), stop=(d == 7))
                            dst = qTp[fc][:, qc * 512:(qc + 1) * 512]
                            if cp_pick() == "A":
                                nc.scalar.activation(dst, ps[:], AF.Copy)
                            else:
                                nc.vector.tensor_copy(dst, ps[:])

                    # ---- K/V exchange: AllReduce(sum) over pairs,
                    # peer = sum - own ----
                    groups = [[0, 1], [2, 3], [4, 5], [6, 7]]
                    with tc.tile_pool(name="ccdram", bufs=1,
                                      space="DRAM") as ccd:
                        ccv_in = ccd.tile([128, 8192], BF16, tag="ccv_in")
                        ccv_out = ccd.tile([128, 8192], BF16, tag="ccv_out")
                        cck_in = ccd.tile([128, 8192], BF16, tag="cck_in")
                        cck_out = ccd.tile([128, 8192], BF16, tag="cck_out")
                        for t in range(8):
                            nc.sync.dma_start(
                                ccv_in[:, t * 512:(t + 1) * 512],
                                vte[t][:, :, 0:64])
                            nc.sync.dma_start(
                                ccv_in[:, 4096 + t * 512:
                                       4096 + (t + 1) * 512],
                                vto[t][:, :, 64:128])
                        nc.gpsimd.collective_compute(
                            "AllReduce", ALU.add, replica_groups=groups,
                            ins=[ccv_in[:].opt()], outs=[ccv_out[:].opt()])
                        for i in range(8):
                            nc.sync.dma_start(
                                cck_in[:, i * 1024:(i + 1) * 1024],
                                kT[i][:, 0:SLOC])
                        nc.gpsimd.collective_compute(
                            "AllReduce", ALU.add, replica_groups=groups,
                            ins=[cck_in[:].opt()], outs=[cck_out[:].opt()])
                        with tc.tile_pool(name="stag", bufs=3) as stag:
                            for i in range(8):
                                ksum = stag.tile([128, 1024], BF16,
                                                 tag="ksum", name="ksum")
                                nc.sync.dma_start(
                                    ksum[:],
                                    cck_out[:, i * 1024:(i + 1) * 1024])
                                nc.gpsimd.tensor_sub(
                                    kT[i][:, SLOC:S], ksum[:],
                                    kT[i][:, 0:SLOC])
                            for t in range(8):
                                vsum = stag.tile([128, 8, 64], BF16,
                                                 tag="vsum", name="vsum")
                                nc.sync.dma_start(
                                    vsum[:],
                                    ccv_out[:, t * 512:(t + 1) * 512]
                                    .rearrange("p (j x) -> p j x", x=64))
                                nc.gpsimd.tensor_sub(
                                    vte[t + 8][:, :, 0:64], vsum[:],
                                    vte[t][:, :, 0:64])
                                vsum2 = stag.tile([128, 8, 64], BF16,
                                                  tag="vsum", name="vsum2")
                                nc.sync.dma_start(
                                    vsum2[:],
                                    ccv_out[:, 4096 + t * 512:
                                            4096 + (t + 1) * 512]
                                    .rearrange("p (j x) -> p j x", x=64))
                                nc.gpsimd.tensor_sub(
                                    vto[t + 8][:, :, 64:128], vsum2[:],
                                    vto[t][:, :, 64:128])

                # ============ attention ============
                attn_ctx = tc.tile_pool(name="attnbuf", bufs=1)
                attn_pool = attn_ctx.__enter__()
                attn = [attn_pool.tile([128, SLOC], BF16, tag=f"attn{i}",
                                       name=f"attn{i}") for i in range(8)]
                with (
                    tc.tile_pool(name="p2", bufs=6) as p2_pool,
                    tc.tile_pool(name="bcs", bufs=3) as bcs_pool,
                    tc.tile_pool(name="sc_ps", bufs=2, space="PSUM") as sc_ps,
                    tc.tile_pool(name="av_ps", bufs=2, space="PSUM") as av_ps,
                    tc.tile_pool(name="bc_ps", bufs=2, space="PSUM") as bc_ps,
                ):
                    for hp in range(8):
                        for qc in range(2):
                            qsl = slice(qc * 512, (qc + 1) * 512)
                            av0 = av_ps.tile([65, 512], F32, tag="av")
                            av1 = av_ps.tile([128, 512], F32, tag="av")
                            for kc in range(16):
                                ksl = slice(kc * 128, (kc + 1) * 128)
                                sc2 = sc_ps.tile([128, 1024], F32, tag="sc")
                                nc.tensor.matmul(
                                    sc2[:, 0:512], kT[hp][0:64, ksl],
                                    qTp[hp][0:64, qsl],
                                    start=True, stop=True,
                                    tile_position=(0, 0))
                                nc.tensor.matmul(
                                    sc2[:, 512:1024], kT[hp][64:128, ksl],
                                    qTp[hp][64:128, qsl],
                                    start=True, stop=True,
                                    tile_position=(64, 0))
                                p2 = p2_pool.tile([128, 1024], BF16, tag="p")
                                mode = exp_pick()
                                if mode == "F":      # full chunk on ACT
                                    nc.scalar.activation(p2[:], sc2[:],
                                                         AF.Exp, scale=0.125)
                                elif mode == "W":    # full chunk on DVE
                                    nc.vector.tensor_scalar(
                                        p2[:].bitcast(I16), sc2[:],
                                        SCH_A, SCH_B, ALU.mult, ALU.add)
                                elif mode == "A":    # ACT even / DVE odd
                                    nc.scalar.activation(
                                        p2[:, 0:512], sc2[:, 0:512],
                                        AF.Exp, scale=0.125)
                                    nc.vector.tensor_scalar(
                                        p2[:, 512:1024].bitcast(I16),
                                        sc2[:, 512:1024],
                                        SCH_A, SCH_B, ALU.mult, ALU.add)
                                else:                # DVE even / ACT odd
                                    nc.vector.tensor_scalar(
                                        p2[:, 0:512].bitcast(I16),
                                        sc2[:, 0:512],
                                        SCH_A, SCH_B, ALU.mult, ALU.add)
                                    nc.scalar.activation(
                                        p2[:, 512:1024], sc2[:, 512:1024],
                                        AF.Exp, scale=0.125)
                                nc.tensor.matmul(
                                    av0[:], vte[kc][:, hp, :], p2[:, 0:512],
                                    start=(kc == 0), stop=(kc == 15),
                                    skip_group_check=True)
                                nc.tensor.matmul(
                                    av1[:], vto[kc][:, hp, :],
                                    p2[:, 512:1024],
                                    start=(kc == 0), stop=(kc == 15),
                                    skip_group_check=True)
                            # normalization
                            R = R_tiles[(hp * 2 + qc) % 2]
                            with nc.allow_low_precision(
                                    reason="softmax recip rounded to f32r"):
                                nc.vector.reciprocal(R[64:65, :],
                                                     av0[64:65, :])
                                nc.vector.reciprocal(R[0:1, :], av1[0:1, :])
                            bc = bc_ps.tile([128, 512], F32, tag="bc")
                            nc.tensor.matmul(bc[:], E[:], R[:], start=True,
                                             stop=True)
                            if os.environ.get("BC_DIRECT", "0") == "1":
                                bc_rd = bc
                            else:
                                bc_rd = bcs_pool.tile([128, 512], F32,
                                                      tag="bcsb", name="bcsb")
                                nc.scalar.activation(bc_rd[:], bc[:], AF.Copy)
                            nc.vector.tensor_mul(attn[hp][0:64, qsl],
                                                 av0[0:64, :], bc_rd[0:64, :])
                            nc.vector.tensor_mul(attn[hp][64:128, qsl],
                                                 av1[64:128, :],
                                                 bc_rd[64:128, :])

                # ============ output projection ============
                with (
                    tc.tile_pool(name="osb", bufs=3) as osb_pool,
                    tc.tile_pool(name="d_ps", bufs=2, space="PSUM") as d_ps,
                ):
                    for qm in range(8):
                        for nf in range(2):
                            nsl = slice(nf * 512, (nf + 1) * 512)
                            ps = d_ps.tile([128, 512], F32, tag="fin")
                            for d in range(8):
                                nc.tensor.matmul(
                                    ps[:], attn[d][:, qm * 128:(qm + 1) * 128],
                                    woT[d][:, nsl],
                                    start=(d == 0), stop=(d == 7))
                            nc.tensor.matmul(ps[:], ones_row[:],
                                             bias_bf[:, nsl], start=False,
                                             stop=False, skip_group_check=True)
                            osb = osb_pool.tile([128, 512], F32, tag="osb")
                            if cp_pick() == "A":
                                nc.scalar.activation(osb[:], ps[:], AF.Copy)
                            else:
                                nc.vector.tensor_copy(osb[:], ps[:])
                            nc.sync.dma_start(
                                out[qm * 128:(qm + 1) * 128, nsl], osb[:])
                attn_ctx.__exit__(None, None, None)

    nc.compile()
    return nc


def _get_nc():
    if "nc" not in _CACHE:
        _CACHE["nc"] = _build()
    return _CACHE["nc"]


def host_prep(query, w_qkv, w_out, b_out):
    bf = ml_dtypes.bfloat16
    wT = np.ascontiguousarray(np.asarray(w_qkv, np.float32).T.astype(bf))
    woT = np.ascontiguousarray(np.asarray(w_out, np.float32).T.astype(bf))
    b_out = np.ascontiguousarray(np.asarray(b_out), dtype=np.float32)
    in_maps = []
    for c in range(N_CORES):
        b, half = divmod(c, 2)
        qloc = np.asarray(query[b], np.float32)[half * SLOC:(half + 1) * SLOC]
        qT = np.ascontiguousarray(qloc.T.astype(bf))
        in_maps.append({"qT": qT, "wT": wT, "woT": woT, "b_out": b_out})
    return in_maps


def kernel(query, key, value, w_qkv, w_out, b_out):
    nc = _get_nc()
    in_maps = host_prep(query, w_qkv, w_out, b_out)
    res = run_bass_kernel_spmd(nc, in_maps, core_ids=list(range(N_CORES)))
    out = np.empty((B, S, D), dtype=np.float32)
    for c in range(N_CORES):
        b, half = divmod(c, 2)
        out[b, half * SLOC:(half + 1) * SLOC] = res.results[c]["out"]
    return out
